# revision 22
# baseline (speedup 1.0000x reference)
"""GAT layer (PyG GATConv-style) on 8 Trainium2 NeuronCores via Bass/Tile.

Strategy (see PLAN.md): nodes partitioned 6250/core, edges assigned by dst
(host dst-sort = the sharding), weights replicated.
Launch 1 computes the shared projection xh = x@W (bf16 table) and the
attention dot-products a_src/a_dst = x@u per node (f32).  Host "halo
exchange": replicates the xh table and expands per-edge a_src/a_dst streams.
Launch 2 does, per 128-dst window: batched row-gather of xh[src] (dma_gather),
per-edge logits -> leaky_relu -> exp, scatter-add via selection-matrix matmul
(which also yields segment sums of exp and of a_edge in extra rhs columns),
self-loop handling per window (identity scatter), post-normalization,
and alpha = exp * gather(1/sums).
"""
import sys
sys.path.insert(0, '/opt/trn_rl_repo')
import numpy as np
import ml_dtypes

import concourse.bass as bass
import concourse.mybir as mybir
from concourse import bacc
from concourse.tile import TileContext
from concourse import bass_utils

# problem constants (hardcoded; kernel.py must be self-contained)
N, E, IN, C, H, ED = 50000, 800000, 256, 128, 2, 16
SLOPE = 0.2
NCORES = 8
NPC = N // NCORES            # 6250 nodes per core
PW = 128                     # dsts per window
NW = (NPC + PW - 1) // PW    # 49 windows/core
NPCP = NW * PW               # 6272 padded
WPB = 2                      # windows per gather block
NB = (NW + WPB - 1) // WPB   # blocks
LOHI = 32768                 # int16 index split for the xh table

F32 = mybir.dt.float32
BF16 = mybir.dt.bfloat16
I16 = mybir.dt.int16
I32 = mybir.dt.int32
BF = ml_dtypes.bfloat16

_cache = {}


# ---------------------------------------------------------------- launch 1
def build_launch1():
    nc = bacc.Bacc("TRN2", target_bir_lowering=False, debug=False,
                   num_devices=NCORES)
    xT = nc.dram_tensor("xT", [IN, NPC], F32, kind="ExternalInput")
    W_sb_in = nc.dram_tensor("W_sb", [128, 512], F32, kind="ExternalInput")
    u4_sb_in = nc.dram_tensor("u4_sb", [128, 8], F32, kind="ExternalInput")
    xhT = nc.dram_tensor("xhT", [IN, NPC], BF16, kind="ExternalOutput")
    a4T = nc.dram_tensor("a4T", [4, NPC], F32, kind="ExternalOutput")

    chunks = []
    c0 = 0
    while c0 < NPC:
        cw = min(512, NPC - c0)
        chunks.append((c0, cw))
        c0 += cw

    with TileContext(nc) as tc:
        with (
            tc.tile_pool(name="const", bufs=1) as constp,
            tc.tile_pool(name="xt", bufs=3) as xtp,
            tc.tile_pool(name="ps", bufs=2, space="PSUM") as psp,
            tc.tile_pool(name="psa", bufs=2, space="PSUM") as psap,
            tc.tile_pool(name="ev", bufs=3) as evp,
        ):
            W_sb = constp.tile([128, 512], F32)
            nc.sync.dma_start(out=W_sb[:], in_=W_sb_in[:])
            u4_sb = constp.tile([128, 8], F32)
            nc.sync.dma_start(out=u4_sb[:], in_=u4_sb_in[:])

            for c0, cw in chunks:
                xt0 = xtp.tile([128, 512], F32, tag="xt0")
                nc.sync.dma_start(out=xt0[:, :cw], in_=xT[0:128, c0:c0 + cw])
                xt1 = xtp.tile([128, 512], F32, tag="xt1")
                nc.sync.dma_start(out=xt1[:, :cw], in_=xT[128:256, c0:c0 + cw])
                xts = [xt0, xt1]

                for hc in range(2):
                    ps = psp.tile([128, 512], F32)
                    for kc in range(2):
                        nc.tensor.matmul(
                            out=ps[:, :cw],
                            lhsT=W_sb[:, kc * 256 + hc * 128:
                                      kc * 256 + hc * 128 + 128],
                            rhs=xts[kc][:, :cw],
                            start=(kc == 0), stop=(kc == 1))
                    ev = evp.tile([128, 512], BF16, tag="ev")
                    nc.vector.tensor_copy(out=ev[:, :cw], in_=ps[:, :cw])
                    nc.sync.dma_start(
                        out=xhT[hc * 128:(hc + 1) * 128, c0:c0 + cw],
                        in_=ev[:, :cw])

                psa = psap.tile([4, 512], F32)
                for kc in range(2):
                    nc.tensor.matmul(
                        out=psa[:, :cw],
                        lhsT=u4_sb[:, kc * 4:(kc + 1) * 4],
                        rhs=xts[kc][:, :cw],
                        start=(kc == 0), stop=(kc == 1))
                eva = evp.tile([4, 512], F32, tag="eva")
                nc.scalar.copy(out=eva[:, :cw], in_=psa[:, :cw])
                nc.sync.dma_start(out=a4T[0:4, c0:c0 + cw], in_=eva[:, :cw])

    nc.compile()
    return nc


# ------------------------------------------------------------ edge template
class Template:
    pass


def build_template(row, col):
    """row/col int64 [E]. Builds the static tile template (shared across
    cores) and per-core slot assignments."""
    t = Template()
    # sort edges so each (core, window, lo/hi) segment is contiguous
    segid0 = ((col // NPC) * NW + (col % NPC) // PW) * 2 + (row >= LOHI)
    perm = np.argsort(segid0, kind='stable')
    col_s = col[perm]
    row_s = row[perm]
    core = col_s // NPC
    win = (col_s % NPC) // PW
    lohi = (row_s >= LOHI).astype(np.int64)
    segid = (core * NW + win) * 2 + lohi          # [E]
    cnts = np.bincount(segid, minlength=NCORES * NW * 2)
    cnts3 = cnts.reshape(NCORES, NW, 2)
    T_lo = np.ceil(cnts3[:, :, 0].max(axis=0) / 128).astype(int)  # [NW]
    T_hi = np.ceil(cnts3[:, :, 1].max(axis=0) / 128).astype(int)

    # block structure (static, shared by all cores)
    blocks = []
    tile_base = 0
    for b in range(NB):
        wins = list(range(b * WPB, min((b + 1) * WPB, NW)))
        tlo = [int(T_lo[w]) for w in wins]
        thi = [int(T_hi[w]) for w in wins]
        BTlo = sum(tlo)
        BT = BTlo + sum(thi)
        lo_off = np.concatenate([[0], np.cumsum(tlo)[:-1]]).astype(int)
        hi_off = np.concatenate([[0], np.cumsum(thi)[:-1]]).astype(int)
        blocks.append(dict(b=b, wins=wins, tlo=tlo, thi=thi, BTlo=BTlo,
                           BT=BT, lo_off=lo_off, hi_off=hi_off,
                           tile_base=tile_base))
        tile_base += BT
    t.blocks = blocks
    t.total_T = tile_base
    t.BTmax = max(bl["BT"] for bl in blocks)
    t.T_lo, t.T_hi = T_lo, T_hi

    # slot base (in tiles) for each (window, lohi): tile col within global
    # tile order
    slot_tile = np.zeros((NW, 2), dtype=np.int64)
    for bl in blocks:
        for i, w in enumerate(bl["wins"]):
            slot_tile[w, 0] = bl["tile_base"] + bl["lo_off"][i]
            slot_tile[w, 1] = bl["tile_base"] + bl["BTlo"] + bl["hi_off"][i]

    # per-core: slot index of every sorted edge
    seg_start = np.zeros(NCORES * NW * 2 + 1, dtype=np.int64)
    np.cumsum(cnts, out=seg_start[1:])
    pos_in_seg = np.arange(E) - seg_start[segid]
    slot_base = slot_tile[:, 0] * 0  # placeholder
    sb = (slot_tile.reshape(-1))      # [NW*2] tiles
    slot_of_edge = sb[win * 2 + lohi] * 128 + pos_in_seg   # slot within core
    t.perm = perm
    t.row_s, t.col_s = row_s, col_s
    t.core_of_edge = core
    t.slot_of_edge = slot_of_edge
    t.core_bounds = np.searchsorted(core, np.arange(NCORES + 1))
    t.S_slots = t.total_T * 128
    return t


def build_core_arrays(t, ea_sorted_all):
    """Per-core per-slot numpy arrays (except a_src/a_dst which need launch-1
    results).  ea_sorted_all = edge_attr[t.perm]."""
    out = []
    for k in range(NCORES):
        b0, b1 = t.core_bounds[k], t.core_bounds[k + 1]
        slots = t.slot_of_edge[b0:b1]
        S = t.S_slots
        src = np.zeros(S, dtype=np.int64)
        dstw = np.full(S, -1.0, dtype=np.float32)
        dstb = np.zeros(S, dtype=np.int32)
        ea = np.zeros((S, ED), dtype=np.float32)
        rs = t.row_s[b0:b1]
        cs = t.col_s[b0:b1]
        src[slots] = rs
        loc = cs - k * NPC
        dstw[slots] = (loc % PW).astype(np.float32)
        ea[slots] = ea_sorted_all[b0:b1]
        # block-local dst (for the r-table gather)
        blk_of_w = np.zeros(NW, dtype=np.int64)
        for bl in t.blocks:
            for w in bl["wins"]:
                blk_of_w[w] = bl["b"]
        dstb[slots] = (loc - blk_of_w[loc // PW] * WPB * PW).astype(np.int32)
        # int16 gather index (lo table or hi table)
        idx16 = np.where(src >= LOHI, src - LOHI, src).astype(np.int16)
        out.append(dict(slots=slots, src=src, idx16=idx16, dstw=dstw,
                        dstb=dstb, ea=ea))
    return out


def _chunkify(arr_slots, inner):
    """[S_slots, inner] -> [128, total_T*inner] (slot (tile,p) -> [p, tile])"""
    T = arr_slots.shape[0] // 128
    a = arr_slots.reshape(T, 128, inner).transpose(1, 0, 2)
    return np.ascontiguousarray(a.reshape(128, T * inner))


def _wrap_idx(seq):
    """int16 idx sequence [n] (n % 128 == 0) -> [128, n//16] wrapped layout"""
    w = seq.reshape(-1, 16).T          # [16, n/16]
    return np.ascontiguousarray(np.tile(w, (8, 1)))   # [128, n/16]


def build_in2_core(t, cores, k, xh, a4, col, bias, v):
    """Input map for core k of launch 2 (xh bf16 [N,256], a4 [N,4])."""
    ca = cores[k]
    S = t.S_slots
    idx_all = _wrap_idx(ca["idx16"])
    ea_dup = np.concatenate([ca["ea"], ca["ea"]], axis=1)
    asrc_pe = a4[ca["src"], 0:2].astype(np.float32)
    dst_glob = np.zeros(S, dtype=np.int64)
    dst_glob[ca["slots"]] = t.col_s[t.core_bounds[k]:t.core_bounds[k + 1]]
    adst_pe = a4[dst_glob, 2:4].astype(np.float32)
    padmask = np.ones(S, dtype=bool)
    padmask[ca["slots"]] = False
    asrc_pe[padmask] = 0.0
    adst_pe[padmask] = 0.0

    own = slice(k * NPC, (k + 1) * NPC)
    xh_own = np.zeros((NPCP, 256), dtype=BF)
    xh_own[:NPC] = xh[own]
    awself = np.zeros((NPCP, 4), dtype=np.float32)
    awself[:NPC] = a4[own]
    deg = np.bincount(col, minlength=N)[own].astype(np.float32)
    icnt = np.ones((NPCP, 1), dtype=np.float32)
    icnt[:NPC, 0] = 1.0 / np.maximum(deg, 1.0)

    return {
        "xh_lo": np.ascontiguousarray(xh[:LOHI]),
        "xh_hi": np.ascontiguousarray(xh[LOHI:]),
        "xh_own": xh_own,
        "idx_all": idx_all,
        "dstw": _chunkify(ca["dstw"].reshape(-1, 1), 1),
        "dstb16": _wrap_idx(ca["dstb"].astype(np.int16)),
        "ea_dup": _chunkify(ea_dup, 2 * ED),
        "asrc_pe": _chunkify(asrc_pe, 2),
        "adst_pe": _chunkify(adst_pe, 2),
        "awself": awself, "icnt": icnt,
        "bias_rep": np.tile(bias, (128, 1)).astype(np.float32),
        "iota_row": np.tile(np.arange(128, dtype=np.float32), (128, 1)),
        "vrep": np.tile(v.T.reshape(-1), (128, 1)).astype(np.float32),
    }


# ---------------------------------------------------------------- launch 2
def build_launch2(t, stage=4):
    nc = bacc.Bacc("TRN2", target_bir_lowering=False, debug=False,
                   num_devices=NCORES)
    TT = t.total_T
    BTmax = t.BTmax

    xh_lo = nc.dram_tensor("xh_lo", [LOHI, 256], BF16, kind="ExternalInput")
    xh_hi = nc.dram_tensor("xh_hi", [N - LOHI, 256], BF16, kind="ExternalInput")
    xh_own = nc.dram_tensor("xh_own", [NPCP, 256], BF16, kind="ExternalInput")
    idx_all = nc.dram_tensor("idx_all", [128, TT * 8], I16, kind="ExternalInput")
    dstw_in = nc.dram_tensor("dstw", [128, TT], F32, kind="ExternalInput")
    dstb_in = nc.dram_tensor("dstb16", [128, TT * 8], I16, kind="ExternalInput")
    ea_in = nc.dram_tensor("ea_dup", [128, TT * 2 * ED], F32, kind="ExternalInput")
    as0_in = nc.dram_tensor("asrc_pe", [128, TT * 2], F32, kind="ExternalInput")
    as1_in = nc.dram_tensor("adst_pe", [128, TT * 2], F32, kind="ExternalInput")
    aws_in = nc.dram_tensor("awself", [NPCP, 4], F32, kind="ExternalInput")
    icnt_in = nc.dram_tensor("icnt", [NPCP, 1], F32, kind="ExternalInput")
    bias_in = nc.dram_tensor("bias_rep", [128, 128], F32, kind="ExternalInput")
    iota_in = nc.dram_tensor("iota_row", [128, 128], F32, kind="ExternalInput")
    vrep_in = nc.dram_tensor("vrep", [128, 2 * ED], F32, kind="ExternalInput")

    outw = nc.dram_tensor("outw", [NPCP, 128], F32, kind="ExternalOutput")
    alpha_d = nc.dram_tensor("alpha_d", [128, TT * 2], F32, kind="ExternalOutput")
    alpha_s = nc.dram_tensor("alpha_s", [128, NW * 2], F32, kind="ExternalOutput")

    # r-table rows padded to 256B so dma_gather (elem 256B min) can expand
    # r to per-edge values by block-local dst index
    rtabs = [nc.dram_tensor(f"rtab{b}", [WPB * PW, 64], F32, kind="Internal")
             for b in range(NB)]

    AF = mybir.ActivationFunctionType

    with TileContext(nc) as tc:
        with (
            tc.tile_pool(name="const", bufs=1) as constp,
            tc.tile_pool(name="gb", bufs=2) as gbp,
            tc.tile_pool(name="idx", bufs=2) as idxp,
            tc.tile_pool(name="stream", bufs=2) as strp,
            tc.tile_pool(name="small", bufs=2) as smp,
            tc.tile_pool(name="S", bufs=2) as Sp,
            tc.tile_pool(name="rhs", bufs=2) as rhsp,
            tc.tile_pool(name="psum", bufs=2, space="PSUM") as psp,
            tc.tile_pool(name="win", bufs=3) as winp,
            tc.tile_pool(name="out", bufs=3) as outp,
        ):
            iota = constp.tile([128, 128], F32)
            nc.sync.dma_start(out=iota[:], in_=iota_in[:])
            vrep = constp.tile([128, 2 * ED], F32)
            nc.sync.dma_start(out=vrep[:], in_=vrep_in[:])
            bias = constp.tile([128, 128], F32)
            nc.sync.dma_start(out=bias[:], in_=bias_in[:])

            for bl in t.blocks:
                b = bl["b"]
                BT, BTlo = bl["BT"], bl["BTlo"]
                BThi = BT - BTlo
                base = bl["tile_base"]

                # ---- gather xh rows for the whole block
                gb = gbp.tile([128, BTmax * 256], BF16, tag="gb")
                gb3 = gb[:, :BT * 256].rearrange("p (t c) -> p t c", c=256)
                idxt = idxp.tile([128, BTmax * 8], I16, tag="idx")
                nc.sync.dma_start(out=idxt[:, :BT * 8],
                                  in_=idx_all[:, base * 8:(base + BT) * 8])
                if BTlo > 0:
                    nc.gpsimd.dma_gather(
                        out_ap=gb3[:, 0:BTlo, :],
                        in_ap=xh_lo[:],
                        idxs_ap=idxt[:, :BTlo * 8],
                        num_idxs=BTlo * 128,
                        num_idxs_reg=BTlo * 128,
                        elem_size=256,
                        single_packet=False,
                    )
                if BThi > 0:
                    nc.gpsimd.dma_gather(
                        out_ap=gb3[:, BTlo:BT, :],
                        in_ap=xh_hi[:],
                        idxs_ap=idxt[:, BTlo * 8:BT * 8],
                        num_idxs=BThi * 128,
                        num_idxs_reg=BThi * 128,
                        elem_size=256,
                        single_packet=False,
                    )

                if stage < 2:
                    # bisect stage 1: just prove the gathers run; touch gb
                    sink = smp.tile([128, 2], F32, tag="sink")
                    nc.vector.tensor_copy(out=sink[:], in_=gb[:, 0:2])
                    nc.sync.dma_start(out=alpha_s[:, 0:2], in_=sink[:])
                    continue

                # ---- per-slot streams
                ea = strp.tile([128, BTmax * 2 * ED], F32, tag="ea")
                nc.sync.dma_start(
                    out=ea[:, :BT * 2 * ED],
                    in_=ea_in[:, base * 2 * ED:(base + BT) * 2 * ED])
                as0 = smp.tile([128, BTmax * 2], F32, tag="as0")
                nc.sync.dma_start(out=as0[:, :BT * 2],
                                  in_=as0_in[:, base * 2:(base + BT) * 2])
                as1 = smp.tile([128, BTmax * 2], F32, tag="as1")
                nc.sync.dma_start(out=as1[:, :BT * 2],
                                  in_=as1_in[:, base * 2:(base + BT) * 2])
                dw = smp.tile([128, BTmax], F32, tag="dw")
                nc.sync.dma_start(out=dw[:, :BT],
                                  in_=dstw_in[:, base:base + BT])
                db = smp.tile([128, BTmax * 8], I16, tag="db")
                nc.sync.dma_start(out=db[:, :BT * 8],
                                  in_=dstb_in[:, base * 8:(base + BT) * 8])

                # ---- a_edge = sum over ED of ea*v per head
                work = strp.tile([128, BTmax * 2 * ED], F32, tag="work")
                w3o = work[:, :BT * 2 * ED].rearrange("p (t c) -> p t c",
                                                      c=2 * ED)
                e3i = ea[:, :BT * 2 * ED].rearrange("p (t c) -> p t c",
                                                    c=2 * ED)
                v3 = vrep[:].rearrange("p (o c) -> p o c", o=1) \
                    .to_broadcast([128, BT, 2 * ED])
                nc.vector.tensor_tensor(out=w3o, in0=e3i, in1=v3,
                                        op=mybir.AluOpType.mult)
                ae = smp.tile([128, BTmax * 2], F32, tag="ae")
                w4 = work[:, :BT * 2 * ED].rearrange("p (g c) -> p g c", c=ED)
                a3 = ae[:, :BT * 2].rearrange("p (g o) -> p g o", o=1)
                nc.vector.reduce_sum(out=a3, in_=w4, axis=mybir.AxisListType.X)

                # ---- logits -> exp
                z = smp.tile([128, BTmax * 2], F32, tag="z")
                nc.vector.tensor_add(out=z[:, :BT * 2], in0=as0[:, :BT * 2],
                                     in1=as1[:, :BT * 2])
                nc.vector.tensor_add(out=z[:, :BT * 2], in0=z[:, :BT * 2],
                                     in1=ae[:, :BT * 2])
                # leaky relu: l = z - (1-slope)*relu(-z)
                ng = smp.tile([128, BTmax * 2], F32, tag="ng")
                nc.scalar.activation(out=ng[:, :BT * 2], in_=z[:, :BT * 2],
                                     func=AF.Relu, scale=-1.0)
                lr = smp.tile([128, BTmax * 2], F32, tag="lr")
                nc.vector.scalar_tensor_tensor(
                    out=lr[:, :BT * 2], in0=ng[:, :BT * 2],
                    scalar=(1.0 - SLOPE), in1=z[:, :BT * 2],
                    op0=mybir.AluOpType.mult, op1=mybir.AluOpType.add)
                ex = smp.tile([128, BTmax * 2], F32, tag="ex")
                nc.scalar.activation(out=ex[:, :BT * 2], in_=lr[:, :BT * 2],
                                     func=AF.Exp)

                # ---- selection matrices for all tiles of the block
                S = Sp.tile([128, BTmax * 128], BF16, tag="S")
                S3 = S[:, :BT * 128].rearrange("p (t c) -> p t c", c=128)
                dw3 = dw[:, :BT].rearrange("p (t o) -> p t o", o=1) \
                    .to_broadcast([128, BT, 128])
                i3 = iota[:].rearrange("p (o c) -> p o c", o=1) \
                    .to_broadcast([128, BT, 128])
                nc.vector.tensor_tensor(out=S3, in0=dw3, in1=i3,
                                        op=mybir.AluOpType.is_equal)

                # ---- rhs build
                rhs = rhsp.tile([128, BTmax * 260], BF16, tag="rhs")
                r3 = rhs[:, :BT * 260].rearrange("p (t c) -> p t c", c=260)
                g3 = gb3
                e3 = ex[:, :BT * 2].rearrange("p (t h) -> p t h", h=2)
                for h in range(2):
                    nc.vector.tensor_tensor(
                        out=r3[:, :, h * 128:(h + 1) * 128],
                        in0=g3[:, :, h * 128:(h + 1) * 128],
                        in1=e3[:, :, h:h + 1].to_broadcast([128, BT, 128]),
                        op=mybir.AluOpType.mult)
                nc.vector.tensor_copy(out=r3[:, :, 256:258], in_=e3)
                ae3 = ae[:, :BT * 2].rearrange("p (t h) -> p t h", h=2)
                nc.vector.tensor_copy(out=r3[:, :, 258:260], in_=ae3)

                if stage < 3:
                    sink = smp.tile([128, 2], F32, tag="sink")
                    nc.vector.tensor_copy(out=sink[:], in_=rhs[:, 0:2])
                    nc.sync.dma_start(out=alpha_s[:, 0:2], in_=sink[:])
                    continue

                # ---- per-window: scatter matmul + self-loop + normalize
                for wi, w in enumerate(bl["wins"]):
                    tiles = (list(range(bl["lo_off"][wi],
                                        bl["lo_off"][wi] + bl["tlo"][wi])) +
                             list(range(BTlo + bl["hi_off"][wi],
                                        BTlo + bl["hi_off"][wi] + bl["thi"][wi])))
                    agg = psp.tile([128, 260], F32)
                    for j, tc_ in enumerate(tiles):
                        nc.tensor.matmul(
                            out=agg[:],
                            lhsT=S[:, tc_ * 128:(tc_ + 1) * 128],
                            rhs=rhs[:, tc_ * 260:(tc_ + 1) * 260],
                            start=(j == 0), stop=(j == len(tiles) - 1))
                    AGG = winp.tile([128, 260], F32, tag="AGG")
                    if tiles:
                        nc.scalar.copy(out=AGG[:], in_=agg[:])
                    else:
                        nc.vector.memset(AGG[:], 0.0)

                    xhs = winp.tile([128, 256], BF16, tag="xhs")
                    nc.sync.dma_start(out=xhs[:],
                                      in_=xh_own[w * 128:(w + 1) * 128, :])
                    aw = winp.tile([128, 4], F32, tag="aw")
                    nc.sync.dma_start(out=aw[:],
                                      in_=aws_in[w * 128:(w + 1) * 128, :])
                    ic = winp.tile([128, 1], F32, tag="ic")
                    nc.sync.dma_start(out=ic[:],
                                      in_=icnt_in[w * 128:(w + 1) * 128, :])

                    sm = winp.tile([128, 18], F32, tag="sm")
                    aem, zsl, lsl, esl = (sm[:, 0:2], sm[:, 2:4],
                                          sm[:, 4:6], sm[:, 6:8])
                    sums, r, r05, asl = (sm[:, 8:10], sm[:, 10:12],
                                         sm[:, 12:14], sm[:, 14:16])
                    ngs = sm[:, 16:18]
                    nc.vector.tensor_tensor(
                        out=aem, in0=AGG[:, 258:260],
                        in1=ic[:].to_broadcast([128, 2]),
                        op=mybir.AluOpType.mult)
                    nc.vector.tensor_add(out=zsl, in0=aw[:, 0:2],
                                         in1=aw[:, 2:4])
                    nc.vector.tensor_add(out=zsl, in0=zsl, in1=aem)
                    nc.scalar.activation(out=ngs, in_=zsl, func=AF.Relu,
                                         scale=-1.0)
                    nc.vector.scalar_tensor_tensor(
                        out=lsl, in0=ngs, scalar=(1.0 - SLOPE), in1=zsl,
                        op0=mybir.AluOpType.mult, op1=mybir.AluOpType.add)
                    nc.scalar.activation(out=esl, in_=lsl, func=AF.Exp)
                    nc.vector.tensor_add(out=sums, in0=AGG[:, 256:258],
                                         in1=esl)
                    nc.vector.reciprocal(out=r, in_=sums)
                    nc.vector.tensor_scalar_mul(out=r05, in0=r, scalar1=0.5)
                    nc.vector.tensor_tensor(out=asl, in0=esl, in1=r,
                                            op=mybir.AluOpType.mult)
                    nc.sync.dma_start(out=alpha_s[:, w * 2:(w + 1) * 2],
                                      in_=asl)
                    nc.sync.dma_start(
                        out=rtabs[b][wi * 128:(wi + 1) * 128, 0:2], in_=r)

                    ow = outp.tile([128, 128], F32, tag="ow")
                    t0 = outp.tile([128, 128], F32, tag="t0")
                    for h in range(2):
                        m = outp.tile([128, 128], F32, tag=f"m{h}")
                        nc.vector.tensor_tensor(
                            out=m[:], in0=xhs[:, h * 128:(h + 1) * 128],
                            in1=esl[:, h:h + 1].to_broadcast([128, 128]),
                            op=mybir.AluOpType.mult)
                        nc.vector.tensor_add(
                            out=m[:], in0=m[:],
                            in1=AGG[:, h * 128:(h + 1) * 128])
                        nc.scalar.mul(out=(t0[:] if h == 0 else m[:]),
                                      in_=m[:], mul=r05[:, h:h + 1])
                        if h == 1:
                            nc.vector.tensor_add(out=t0[:], in0=t0[:],
                                                 in1=m[:])
                    nc.vector.tensor_add(out=ow[:], in0=t0[:], in1=bias[:])
                    nc.sync.dma_start(out=outw[w * 128:(w + 1) * 128, :],
                                      in_=ow[:])

                if stage < 4:
                    continue

                # ---- alpha for the block's edge slots: expand r by
                # block-local dst via dma_gather of the padded r-table
                rexp = strp.tile([128, BTmax * 64], F32, tag="rexp")
                rexp3 = rexp[:, :BT * 64].rearrange("p (t c) -> p t c", c=64)
                nc.gpsimd.dma_gather(
                    out_ap=rexp3,
                    in_ap=rtabs[b][:],
                    idxs_ap=db[:, :BT * 8],
                    num_idxs=BT * 128,
                    num_idxs_reg=BT * 128,
                    elem_size=64,
                    single_packet=False,
                )
                al = strp.tile([128, BTmax * 2], F32, tag="al")
                nc.vector.tensor_tensor(out=al[:, :BT * 2].rearrange(
                                            "p (t h) -> p t h", h=2),
                                        in0=e3,
                                        in1=rexp3[:, :, 0:2],
                                        op=mybir.AluOpType.mult)
                nc.sync.dma_start(out=alpha_d[:, base * 2:(base + BT) * 2],
                                  in_=al[:, :BT * 2])

    nc.compile()
    return nc


# ------------------------------------------------------------------ driver
def kernel(x, edge_index, edge_attr, W, W_edge, att_src, att_dst, att_edge,
           bias, _collect_timing=None):
    x = np.asarray(x, dtype=np.float32)
    ei_dtype = np.asarray(edge_index).dtype
    row = np.asarray(edge_index[0], dtype=np.int64)
    col = np.asarray(edge_index[1], dtype=np.int64)
    edge_attr = np.asarray(edge_attr, dtype=np.float32)
    W = np.asarray(W, dtype=np.float32)
    W_edge = np.asarray(W_edge, dtype=np.float32)
    att_src = np.asarray(att_src, dtype=np.float32)
    att_dst = np.asarray(att_dst, dtype=np.float32)
    att_edge = np.asarray(att_edge, dtype=np.float32)
    bias = np.asarray(bias, dtype=np.float32)

    # ---------------- weight folding (host, weights only)
    u_src = np.stack([W[:, h * C:(h + 1) * C] @ att_src[h] for h in range(H)],
                     axis=1)
    u_dst = np.stack([W[:, h * C:(h + 1) * C] @ att_dst[h] for h in range(H)],
                     axis=1)
    u4 = np.concatenate([u_src, u_dst], axis=1).astype(np.float32)
    v = np.stack([W_edge[:, h * C:(h + 1) * C] @ att_edge[h]
                  for h in range(H)], axis=1).astype(np.float32)  # [ED, 2]
    W_sb = W.reshape(2, 128, 256).transpose(1, 0, 2).reshape(128, 512)
    W_sb = np.ascontiguousarray(W_sb, dtype=np.float32)
    u4_sb = np.ascontiguousarray(
        u4.reshape(2, 128, 4).transpose(1, 0, 2).reshape(128, 8))

    # ---------------- launch 1
    if "l1" not in _cache:
        _cache["l1"] = build_launch1()
    nc1 = _cache["l1"]
    xT = np.ascontiguousarray(x.T)
    in1 = [{"xT": np.ascontiguousarray(xT[:, k * NPC:(k + 1) * NPC]),
            "W_sb": W_sb, "u4_sb": u4_sb} for k in range(NCORES)]
    res1 = bass_utils.run_bass_kernel_spmd(nc1, in1,
                                           core_ids=list(range(NCORES)))
    xh = np.concatenate([res1.results[k]["xhT"].T for k in range(NCORES)],
                        axis=0)                      # [N, 256] bf16
    a4 = np.concatenate([res1.results[k]["a4T"].T for k in range(NCORES)],
                        axis=0)                      # [N, 4] f32
    xh = np.ascontiguousarray(xh)

    # ---------------- host glue: template + per-core arrays
    import hashlib
    tkey = ("tmpl", hashlib.md5(row.tobytes() + col.tobytes()).hexdigest())
    if tkey not in _cache:
        _cache[tkey] = build_template(row, col)
        _cache.pop("l2", None)  # launch-2 NEFF is template-specific
    t = _cache[tkey]
    ea_sorted = edge_attr[t.perm]
    cores = build_core_arrays(t, ea_sorted)

    xh_lo = np.ascontiguousarray(xh[:LOHI])
    xh_hi = np.ascontiguousarray(xh[LOHI:])
    iota_row = np.tile(np.arange(128, dtype=np.float32), (128, 1))
    vrep = np.tile(v.T.reshape(-1), (128, 1)).astype(np.float32)  # [128, 32]
    # careful: v.T.reshape(-1) = [v[:,0] (16) | v[:,1] (16)] matching
    # ea_dup = [ea | ea]
    bias_rep = np.tile(bias, (128, 1)).astype(np.float32)

    if "l2" not in _cache:
        _cache["l2"] = build_launch2(t)
    nc2 = _cache["l2"]

    in2 = []
    for k in range(NCORES):
        ca = cores[k]
        S = t.S_slots
        # gather idx in wrapped layout
        idx_all = _wrap_idx(ca["idx16"])            # [128, S/16] -> per-tile 8
        ea_dup = np.concatenate([ca["ea"], ca["ea"]], axis=1)  # [S, 32]
        asrc_pe = a4[ca["src"], 0:2].astype(np.float32)
        # pads have src=0 -> harmless, masked by S-matrix
        dst_glob = np.zeros(S, dtype=np.int64)
        dst_glob[ca["slots"]] = t.col_s[t.core_bounds[k]:t.core_bounds[k + 1]]
        adst_pe = a4[dst_glob, 2:4].astype(np.float32)
        # zero the pads (so z=0 -> exp=1, finite)
        padmask = np.ones(S, dtype=bool)
        padmask[ca["slots"]] = False
        asrc_pe[padmask] = 0.0
        adst_pe[padmask] = 0.0

        own = slice(k * NPC, (k + 1) * NPC)
        xh_own = np.zeros((NPCP, 256), dtype=BF)
        xh_own[:NPC] = xh[own]
        awself = np.zeros((NPCP, 4), dtype=np.float32)
        awself[:NPC, 0:2] = a4[own, 0:2]
        awself[:NPC, 2:4] = a4[own, 2:4]
        deg = np.bincount(col, minlength=N)[own].astype(np.float32)
        icnt = np.zeros((NPCP, 1), dtype=np.float32)
        icnt[:NPC, 0] = 1.0 / np.maximum(deg, 1.0)
        icnt[NPC:, 0] = 1.0

        in2.append({
            "xh_lo": xh_lo, "xh_hi": xh_hi, "xh_own": xh_own,
            "idx_all": idx_all,
            "dstw": _chunkify(ca["dstw"].reshape(-1, 1), 1),
            "dstb16": _wrap_idx(ca["dstb"].astype(np.int16)),
            "ea_dup": _chunkify(ea_dup, 2 * ED),
            "asrc_pe": _chunkify(asrc_pe, 2),
            "adst_pe": _chunkify(adst_pe, 2),
            "awself": awself, "icnt": icnt,
            "bias_rep": bias_rep, "iota_row": iota_row, "vrep": vrep,
        })

    trace = bool(_collect_timing.get("trace")) if _collect_timing else False
    res2 = bass_utils.run_bass_kernel_spmd(nc2, in2,
                                           core_ids=list(range(NCORES)),
                                           trace=trace)

    # ---------------- unpack
    out = np.concatenate(
        [res2.results[k]["outw"][:NPC] for k in range(NCORES)], axis=0)

    alpha = np.empty((E + N, 2), dtype=np.float32)
    for k in range(NCORES):
        ad = res2.results[k]["alpha_d"]          # [128, TT*2]
        per_slot = ad.reshape(128, t.total_T, 2).transpose(1, 0, 2) \
            .reshape(t.S_slots, 2)
        b0, b1 = t.core_bounds[k], t.core_bounds[k + 1]
        alpha[t.perm[b0:b1]] = per_slot[cores[k]["slots"]]
        asl = res2.results[k]["alpha_s"]         # [128, NW*2]
        per_dst = asl.reshape(128, NW, 2).transpose(1, 0, 2) \
            .reshape(NPCP, 2)
        alpha[E + k * NPC: E + (k + 1) * NPC] = per_dst[:NPC]

    ar = np.arange(N, dtype=ei_dtype)
    ei = np.stack([np.concatenate([row.astype(ei_dtype), ar]),
                   np.concatenate([col.astype(ei_dtype), ar])])

    if _collect_timing is not None:
        _collect_timing["res1"] = res1
        _collect_timing["res2"] = res2
    return out, (ei, alpha)
